# revision 6
# baseline (speedup 1.0000x reference)
"""Constrained-KNN retriever kernel for Trainium2 (8 NeuronCores).

Strategy: shard the candidate pool along N across the 8 cores. Each core
computes masked cosine sims for all T=512 tokens against its shard via
TensorE GEMMs and reduces to per-(token, 512-candidate-tile) (max, argmax)
pairs on VectorE. The host combines the per-core candidates and gathers
the winning feature rows.

Masking is folded into the GEMM: one-hot label encodings contracted with
+-2^100 penalty coefficients (exactly representable in bf16; sums of them
are exact in fp32 PSUM accumulation, so unmasked entries get penalty
exactly 0).

Precision: fp32 features are split into bf16 hi+lo on device; sims are
computed as hi@hi + hi@lo + lo@hi accumulated in fp32 PSUM, giving
~3e-7 absolute error vs the fp32 reference — far below the minimum
top1-top2 sim gap, so the argmax matches the reference exactly.
"""

import numpy as np

import concourse.bacc as bacc
import concourse.mybir as mybir
from concourse import bass_utils
from concourse.tile import TileContext

# Problem constants (hardcoded per contract: kernel.py is self-contained).
T = 512            # tokens
D = 768            # feature dim
N_FULL = 200000    # candidate pool size
N_CORES = 8
N_SHARD = N_FULL // N_CORES          # 25000
N_PAD = 25088                        # 49 * 512 = 196 * 128
N_TILE = 512                         # candidates per GEMM tile (1 PSUM bank)
BLK_TILES = 4                        # n-tiles per block (2048 candidates)
KCH = D // 128                       # 6 contraction chunks for features
PEN_ROWS = 256                       # penalty contraction dims (2 chunks)
PEN = float(2.0 ** 100)              # penalty magnitude (bf16/fp32 exact)

FP32 = mybir.dt.float32
BF16 = mybir.dt.bfloat16
U32 = mybir.dt.uint32
AF = mybir.ActivationFunctionType

_PROGRAM_CACHE = {}


def build_program(n_pad=N_PAD):
    """Build the per-core bass program. Same program runs on all cores
    (SPMD); the shard is chosen purely by the per-core input maps."""
    assert n_pad % N_TILE == 0 and n_pad % 128 == 0
    n_tiles = n_pad // N_TILE
    nc = bacc.Bacc(None, target_bir_lowering=False)

    feat = nc.dram_tensor("feat", [n_pad, D], FP32, kind="ExternalInput")
    hmat = nc.dram_tensor("hmat", [T, D], FP32, kind="ExternalInput")
    pen_pool = nc.dram_tensor("pen_pool", [PEN_ROWS, n_pad], BF16,
                              kind="ExternalInput")
    pen_tok = nc.dram_tensor("pen_tok", [PEN_ROWS, T], BF16,
                             kind="ExternalInput")
    val8 = nc.dram_tensor("val8", [T, n_tiles * 8], FP32,
                          kind="ExternalOutput")
    idx8 = nc.dram_tensor("idx8", [T, n_tiles * 8], U32,
                          kind="ExternalOutput")

    n_tb = T // 128  # 4 token blocks

    with TileContext(nc) as tc:
        with (
            tc.tile_pool(name="persist", bufs=1) as persist,
            tc.tile_pool(name="nat", bufs=3) as nat,
            tc.tile_pool(name="natsc", bufs=3) as natsc,
            tc.tile_pool(name="tr", bufs=2) as tr,
            tc.tile_pool(name="sim", bufs=4) as simp,
            tc.tile_pool(name="ps", bufs=8, space="PSUM") as psp,
        ):
            # Persistent SBUF tiles (bufs=1 pool, distinct tags).
            hT_hi = persist.tile([128, KCH * T], BF16, tag="hT_hi")
            hT_lo = persist.tile([128, KCH * T], BF16, tag="hT_lo")
            ptok_sb = persist.tile([128, 2 * T], BF16, tag="ptok_sb")
            val_bufs = [persist.tile([128, n_tiles * 8], FP32,
                                     tag=f"valb{tb}", name=f"valb{tb}")
                        for tb in range(n_tb)]
            idx_bufs = [persist.tile([128, n_tiles * 8], U32,
                                     tag=f"idxb{tb}", name=f"idxb{tb}")
                        for tb in range(n_tb)]
            # token-side penalty: [256, T] -> [128, 2*T] (chunk c at c*T)
            nc.gpsimd.dma_start(ptok_sb[:, 0:T], pen_tok[0:128, :])
            nc.gpsimd.dma_start(ptok_sb[:, T:2 * T], pen_tok[128:256, :])

            def normalize_split_transpose(src_dram, r0, dst_hi, dst_lo, c0):
                """Load [128, D] fp32 rows r0.., normalize, split to bf16
                hi/lo, DMA-transpose into dst_{hi,lo}[:, k*stride + c0…]."""
                raw = nat.tile([128, D], FP32, tag="raw")
                nc.gpsimd.dma_start(raw[:], src_dram[r0:r0 + 128, :])
                sq = nat.tile([128, D], FP32, tag="sq")
                ss = natsc.tile([128, 1], FP32, tag="ss")
                nc.scalar.activation(sq[:], raw[:], AF.Square, accum_out=ss[:])
                nrm = natsc.tile([128, 1], FP32, tag="nrm")
                nc.scalar.activation(nrm[:], ss[:], AF.Sqrt)
                nc.vector.tensor_scalar_add(nrm[:], nrm[:], 1e-8)
                inv = natsc.tile([128, 1], FP32, tag="inv")
                nc.vector.reciprocal(inv[:], nrm[:])
                fn = nat.tile([128, D], FP32, tag="fn")
                nc.scalar.activation(fn[:], raw[:], AF.Copy, scale=inv[:])
                fhi = nat.tile([128, D], BF16, tag="fhi")
                nc.vector.tensor_copy(fhi[:], fn[:])
                flo = nat.tile([128, D], BF16, tag="flo")
                nc.vector.tensor_sub(flo[:], fn[:], fhi[:])
                for k in range(KCH):
                    for buf, dst in ((fhi, dst_hi), (flo, dst_lo)):
                        nc.sync.dma_start(
                            out=dst[0][:, dst[1] * k + c0:dst[1] * k + c0 + 128],
                            in_=buf[:, k * 128:(k + 1) * 128],
                            transpose=True,
                        )

            # h: normalize + split + transpose into persistent tiles.
            for tb in range(n_tb):
                normalize_split_transpose(hmat, tb * 128,
                                          (hT_hi, T), (hT_lo, T), tb * 128)

            # Main loop over candidate blocks.
            blocks = []
            t0 = 0
            while t0 < n_tiles:
                blocks.append((t0, min(BLK_TILES, n_tiles - t0)))
                t0 += BLK_TILES

            for (tile0, ntiles) in blocks:
                bw = ntiles * N_TILE  # block width in candidates
                n0 = tile0 * N_TILE
                fT_hi = tr.tile([128, KCH * bw], BF16, tag="fT_hi")
                fT_lo = tr.tile([128, KCH * bw], BF16, tag="fT_lo")
                penP = tr.tile([128, 2 * bw], BF16, tag="penP")
                for c in range(2):
                    nc.gpsimd.dma_start(
                        penP[:, c * bw:(c + 1) * bw],
                        pen_pool[c * 128:(c + 1) * 128, n0:n0 + bw])
                for rt in range(bw // 128):
                    normalize_split_transpose(
                        feat, n0 + rt * 128,
                        (fT_hi, bw), (fT_lo, bw), rt * 128)

                for tb in range(n_tb):
                    psum = [psp.tile([128, N_TILE], FP32, tag="ps",
                                     name=f"ps{tb}_{nt}")
                            for nt in range(ntiles)]
                    # penalty MMs first: their +-2^100 terms cancel exactly
                    # before the small sim terms accumulate.
                    for c in range(2):
                        for nt in range(ntiles):
                            nc.tensor.matmul(
                                psum[nt][:],
                                ptok_sb[:, c * T + tb * 128:c * T + (tb + 1) * 128],
                                penP[:, c * bw + nt * N_TILE:c * bw + (nt + 1) * N_TILE],
                                start=(c == 0), stop=False,
                                skip_group_check=True,
                            )
                    combos = ((hT_hi, fT_hi), (hT_hi, fT_lo), (hT_lo, fT_hi))
                    for ci, (hb, fb) in enumerate(combos):
                        for k in range(KCH):
                            last = (ci == 2 and k == KCH - 1)
                            for nt in range(ntiles):
                                nc.tensor.matmul(
                                    psum[nt][:],
                                    hb[:, k * T + tb * 128:k * T + (tb + 1) * 128],
                                    fb[:, k * bw + nt * N_TILE:k * bw + (nt + 1) * N_TILE],
                                    start=False, stop=last,
                                    skip_group_check=True,
                                )
                    for nt in range(ntiles):
                        sim = simp.tile([128, N_TILE], FP32, tag="sim")
                        nc.vector.tensor_copy(sim[:], psum[nt][:])
                        j8 = (tile0 + nt) * 8
                        nc.vector.max(val_bufs[tb][:, j8:j8 + 8], sim[:])
                        nc.vector.max_index(idx_bufs[tb][:, j8:j8 + 8],
                                            val_bufs[tb][:, j8:j8 + 8], sim[:])

            for tb in range(n_tb):
                nc.gpsimd.dma_start(val8[tb * 128:(tb + 1) * 128, :],
                                    val_bufs[tb][:])
                nc.gpsimd.dma_start(idx8[tb * 128:(tb + 1) * 128, :],
                                    idx_bufs[tb][:])

    nc.compile()
    return nc


def get_program(n_pad=N_PAD):
    if n_pad not in _PROGRAM_CACHE:
        _PROGRAM_CACHE[n_pad] = build_program(n_pad)
    return _PROGRAM_CACHE[n_pad]


def build_host_inputs(h_clean, features, genders, phones_pool, symbols_pool,
                      phones, symbols, target_gender, k, n_pad=N_PAD,
                      n_cores=N_CORES):
    """Host-side prep: shard + pad the pool, build penalty matrices."""
    tg, kk = int(target_gender), int(k)
    h = np.ascontiguousarray(h_clean, dtype=np.float32)
    f = np.ascontiguousarray(features, dtype=np.float32)
    g = np.asarray(genders)
    pp = np.asarray(phones_pool)
    sp = np.asarray(symbols_pool)
    ph = np.asarray(phones)
    sy = np.asarray(symbols)
    n = f.shape[0]
    t = h.shape[0]
    shard = n // n_cores
    assert shard * n_cores == n and shard <= n_pad

    base = (g == tg)
    # Global label histograms -> per-token constraint flags (control
    # metadata only; the [T, N] mask itself is applied on-device).
    hist_p = np.zeros(64, np.int64)
    np.add.at(hist_p, pp[base], 1)
    hist_s = np.zeros(100, np.int64)
    np.add.at(hist_s, sp[base], 1)
    joint = np.zeros((64, 100), np.int64)
    np.add.at(joint, (pp[base], sp[base]), 1)

    ph_c = np.clip(ph, 0, 63)
    sy_c = np.clip(sy, 0, 99)
    cnt1 = hist_p[ph_c]
    use_p = cnt1 >= kk
    cnt2 = np.where(use_p, joint[ph_c, sy_c], hist_s[sy_c])
    use_s = (sy >= 0) & (cnt2 >= kk)

    bf16 = mybir.dt.np(BF16)

    ptok = np.zeros((PEN_ROWS, t), np.float32)
    ptok[0, :] = 1.0
    ptok[1 + ph_c, np.arange(t)] = np.where(use_p, PEN, 0.0)
    ptok[65 + sy_c, np.arange(t)] = np.where(use_s, PEN, 0.0)
    ptok[165, :] = -(use_p.astype(np.float64)
                     + use_s.astype(np.float64)) * PEN
    ptok_bf = ptok.astype(bf16)

    in_maps = []
    for c in range(n_cores):
        lo = c * shard
        fs = f[lo:lo + shard]
        fpad = np.ones((n_pad, D), np.float32)
        fpad[:shard] = fs

        pool = np.zeros((PEN_ROWS, n_pad), np.float32)
        pool[0, :] = -PEN
        pool[0, :shard] = np.where(base[lo:lo + shard], 0.0, -PEN)
        cols = np.arange(shard)
        pool[1 + np.clip(pp[lo:lo + shard], 0, 63), cols] = 1.0
        pool[65 + np.clip(sp[lo:lo + shard], 0, 99), cols] = 1.0
        pool[165, :] = 1.0

        in_maps.append({
            "feat": fpad,
            "hmat": h,
            "pen_pool": pool.astype(bf16),
            "pen_tok": ptok_bf,
        })
    return in_maps


def combine(results, features, n_pad=N_PAD, n_cores=N_CORES):
    """Host-side unshard: pick the global argmax per token from the
    per-core per-tile (max, idx) candidates, then gather feature rows."""
    n = features.shape[0]
    shard = n // n_cores
    n_tiles = n_pad // N_TILE
    t = results[0]["val8"].shape[0]
    # [cores, T, n_tiles] top-1 value and within-tile index
    vals = np.stack([r["val8"].reshape(t, n_tiles, 8)[:, :, 0]
                     for r in results])
    idxs = np.stack([r["idx8"].reshape(t, n_tiles, 8)[:, :, 0]
                     for r in results]).astype(np.int64)
    # [T, cores*n_tiles]; argmax picks the first (lowest core, lowest
    # tile) on exact ties, matching np.argmax over the full pool.
    vflat = vals.transpose(1, 0, 2).reshape(t, -1)
    iflat = idxs.transpose(1, 0, 2).reshape(t, -1)
    best = np.argmax(vflat, axis=1)
    core = best // n_tiles
    tile = best % n_tiles
    inner = iflat[np.arange(t), best]
    gidx = core * shard + tile * N_TILE + inner
    return gidx, np.ascontiguousarray(features[gidx])


def run_device(in_maps, n_pad=N_PAD, trace=False, **kwargs):
    nc = get_program(n_pad)
    return bass_utils.run_bass_kernel_spmd(
        nc, in_maps, core_ids=list(range(len(in_maps))), trace=trace,
        **kwargs)


def kernel(h_clean, features, genders, phones_pool, symbols_pool,
           phones, symbols, target_gender, k):
    features = np.ascontiguousarray(features, dtype=np.float32)
    in_maps = build_host_inputs(h_clean, features, genders, phones_pool,
                                symbols_pool, phones, symbols,
                                target_gender, k)
    res = run_device(in_maps)
    _, out = combine(res.results, features)
    return out
